# revision 20
# baseline (speedup 1.0000x reference)
"""Binary dense layer  y = x @ sign(W) + b  on 8 Trainium2 NeuronCores.

Problem (hardcoded): x [4096, 4096] f32, W [4096, 4096] f32, b [4096] f32.

Sharding: 2D grid, 4 batch shards x 2 column shards (one core each).
Per core:  y[1024, 2048] = x[1024, 4096] @ sign(W[4096, 2048]) + b[2048].

Mixed-precision split-K: the 4096 contraction is split K8=2304 (x cast to
fp8 E4M3, matmuls in fp8 DoubleRow mode -- 2 fp8 weights per PE cell,
256-deep contraction per instruction) + K16=1792 (x cast to fp16, normal
matmuls). sign(W) = +-1 is exact in both dtypes and products accumulate
exactly in f32 PSUM, so the only quantization error is the fp8 rounding
of x over K8 of the contraction: measured rel err 1.991e-2 (< 2e-2 gate;
fp8-only would be 2.65e-2, fp16-only 2.1e-4 but ~1.3x slower). The fp8
rounding happens on the host, so the device result matches the numpy
prediction of the error to ~6 digits.

Wire formats (host-side layout prep): x is pre-transposed to K-major and
pre-cast (fp8/fp16 -- the same casts the device would do); W ships as its
sign-carrying high bytes (f32 top byte, u8) for the fp8 part and bf16 for
the fp16 part; both are binarized to +-1 on device with fused DVE bitwise
ops. Bias is pre-broadcast to [128, n]. Host gathers the 8 output shards.
"""

import ml_dtypes
import numpy as np

import concourse.bass as bass
import concourse.mybir as mybir
import concourse.tile as tile
from concourse import bacc, bass_utils
from concourse.bass import ds

# ---- problem constants (fixed by the task; kernel.py must be self-contained)
B_FULL = 4096  # batch rows of x
K_FULL = 4096  # contraction dim (n_in)
N_FULL = 4096  # output cols (n_units)
R, C = 4, 2  # batch shards x column shards -> R*C = 8 cores
N_CORES = 8
P = 128

K8 = 2304  # contraction columns computed in fp8 E4M3 (DoubleRow)
K16 = K_FULL - K8  # contraction columns computed in fp16


def _chunks(n_blocks, target):
    out, lo = [], 0
    while lo < n_blocks:
        sz = min(target, n_blocks - lo)
        out.append((lo, sz))
        lo += sz
    return out


def build_nc(m_loc=B_FULL // R, k8=K8, k16=K16, n_loc=N_FULL // C,
             n_tile=512):
    """Build + compile the per-core Bass kernel (SPMD: same NEFF on all cores).

    y[m_loc, n_loc] = x[m_loc, :] @ sign(W[:, n_loc]) + b[n_loc]

    Loop order is k-outer within each n-tile: all m_tile psum groups
    accumulate in lockstep over k-chunks (fp8 DoubleRow chunks first, then
    fp16 chunks), so during the prologue the PE computes on each arriving
    x k-chunk + W k-slice immediately instead of waiting for whole shards.
    """
    ko8_n = k8 // P
    ko16_n = k16 // P
    m_tiles = m_loc // P
    n_tiles = n_loc // n_tile
    chunks8 = _chunks(ko8_n, 6)   # fp8 chunk sizes must be even (DR pairs)
    # any short remainder chunk goes FIRST: the LAST fp16 chunk runs the
    # per-mt epilogue (bias-add + store), which needs a full 4 matmuls per
    # m-tile to hide the DVE adds behind
    sizes16 = sorted((sz for _, sz in _chunks(ko16_n, 4)))
    chunks16, lo = [], 0
    for sz in sizes16:
        chunks16.append((lo, sz))
        lo += sz
    c8max = max(sz for _, sz in chunks8)
    c16max = max(sz for _, sz in chunks16)

    nc = bacc.Bacc("TRN2", target_bir_lowering=False, debug=False)

    # wire formats are partition-major (host pre-swizzled) so each DMA row
    # is a long contiguous run -> few, large DMA descriptors
    xT8 = nc.dram_tensor("xT8", [P, ko8_n, m_loc], mybir.dt.float8e4,
                         kind="ExternalInput")
    xT16 = nc.dram_tensor("xT16", [P, ko16_n, m_loc], mybir.dt.float16,
                          kind="ExternalInput")
    # W's fp8 half arrives as the top byte of each f32 (sign + 7 exponent
    # bits): sign-preserving and 1/4 the DMA bytes. The fp16 half arrives
    # as bf16 (sign-preserving cast).
    w8 = nc.dram_tensor("w8", [P, n_tiles, ko8_n, n_tile], mybir.dt.uint8,
                        kind="ExternalInput")
    w16 = nc.dram_tensor("w16", [P, n_tiles, ko16_n, n_tile],
                         mybir.dt.bfloat16, kind="ExternalInput")
    bb = nc.dram_tensor("bias", [P, n_loc], mybir.dt.float32,
                        kind="ExternalInput")
    y = nc.dram_tensor("y", [m_loc, n_loc], mybir.dt.float32,
                       kind="ExternalOutput")

    x8_3 = xT8.ap()
    x16_3 = xT16.ap()
    w8_4 = w8.ap()
    w16_4 = w16.ap()
    # output view: row index (mo*P + p) -> [p, mo, n]
    y3 = y.ap().rearrange("(mo p) n -> p mo n", p=P)

    DR = mybir.MatmulPerfMode.DoubleRow

    with tile.TileContext(nc) as tc:
        with (
            tc.tile_pool(name="x_res", bufs=1) as x_res_pool,
            tc.tile_pool(name="stage8", bufs=4) as stage8_pool,
            tc.tile_pool(name="stage16", bufs=4) as stage16_pool,
            tc.tile_pool(name="wq8", bufs=2) as wq8_pool,
            tc.tile_pool(name="wq16", bufs=2) as wq16_pool,
            tc.tile_pool(name="bias_sb", bufs=1) as bias_pool,
            tc.tile_pool(name="yout", bufs=4) as out_pool,
            tc.tile_pool(name="psum", bufs=8, space="PSUM") as psum_pool,
        ):
            # resident x shards, K on partitions
            xt8 = x_res_pool.tile([P, ko8_n, m_loc], mybir.dt.float8e4,
                                  name="xt8")
            xt16 = x_res_pool.tile([P, ko16_n, m_loc], mybir.dt.float16,
                                   name="xt16")

            def load_x8_krange(ko_lo, kos):
                nc.sync.dma_start(
                    xt8[:, ds(ko_lo, kos), :], x8_3[:, ds(ko_lo, kos), :])

            def load_x16_krange(ko_lo, kos):
                nc.sync.dma_start(
                    xt16[:, ds(ko_lo, kos), :], x16_3[:, ds(ko_lo, kos), :])

            def load_w8_krange(wq, nt, ko_lo, kos):
                # ko-range [P, kos, n_tile] u8 of the nt-th W column tile
                wstage = stage8_pool.tile([P, c8max, n_tile],
                                          mybir.dt.uint8, tag="w8stage",
                                          name=f"w8s{nt}_{ko_lo}")
                wst = wstage[:, :kos, :]
                nc.sync.dma_start(wst, w8_4[:, nt, ds(ko_lo, kos), :])
                # DVE fused bitwise binarize on u16 views (bit-parallel over
                # byte pairs): (w & 0x8080) | 0x3838 == +-1.0 fp8e4 per byte.
                dst = wq[:, ds(ko_lo, kos), :]
                nc.vector.tensor_scalar(
                    dst.bitcast(mybir.dt.uint16),
                    wst.bitcast(mybir.dt.uint16),
                    0x8080, 0x3838,
                    mybir.AluOpType.bitwise_and,
                    mybir.AluOpType.bitwise_or)

            def load_w16_krange(wq, nt, ko_lo, kos):
                wstage = stage16_pool.tile([P, c16max, n_tile],
                                           mybir.dt.bfloat16, tag="w16stage",
                                           name=f"w16s{nt}_{ko_lo}")
                wst = wstage[:, :kos, :]
                nc.sync.dma_start(wst, w16_4[:, nt, ds(ko_lo, kos), :])
                # (w & 0x8000) | 0x3C00 on u16 views == +-1.0 fp16
                dst = wq[:, ds(ko_lo, kos), :]
                nc.vector.tensor_scalar(
                    dst.bitcast(mybir.dt.uint16),
                    wst.bitcast(mybir.dt.uint16),
                    0x8000, 0x3C00,
                    mybir.AluOpType.bitwise_and,
                    mybir.AluOpType.bitwise_or)

            # PE warmup: scratch matmuls keep the PE busy through the DMA
            # prologue so the HAM clock gate flips to 8/8 as early as
            # possible (a free-running ~3.4us activity window); just enough
            # of them to bridge until the first real operands land -- every
            # extra warmup matmul delays the PE queue by ~0.5us of cold rate
            n_warm = 7
            scratch = x_res_pool.tile([P, n_tile], mybir.dt.float16,
                                      name="warm_scratch")
            nc.vector.memset(scratch[:], 0.0)
            ps_warm = psum_pool.tile([P, n_tile], mybir.dt.float32, tag="ps",
                                     name="ps_warm")
            for i in range(n_warm):
                nc.tensor.matmul(ps_warm[:], scratch[:, :P], scratch[:],
                                 start=(i == 0), stop=(i == n_warm - 1))

            # prologue: interleave x k-ranges with W tile 0 k-slices in
            # exactly the order the k-outer loop consumes them; the first
            # slice is split so the first real matmul starts sooner.
            # W goes first in each pair: its consumer chain (DMA ->
            # binarize -> MM) is longer than x's (DMA -> MM), and HWDGE
            # DMAs drain in FIFO order.
            wq8_tiles = {0: wq8_pool.tile([P, ko8_n, n_tile],
                                          mybir.dt.float8e4, tag="wq8",
                                          name="wq8_0")}
            wq16_tiles = {0: wq16_pool.tile([P, ko16_n, n_tile],
                                            mybir.dt.float16, tag="wq16",
                                            name="wq16_0")}
            lo0, sz0 = chunks8[0]
            ranges8 = [(lo0, 2), (lo0 + 2, sz0 - 2)] + chunks8[1:]
            for idx, (ko_lo, kos) in enumerate(ranges8):
                load_w8_krange(wq8_tiles[0], 0, ko_lo, kos)
                if idx == 0:
                    # split the first x range by m so the very first matmul
                    # (m-tile 0) only waits on a 32 KB transfer
                    nc.sync.dma_start(xt8[:, ds(ko_lo, kos), :P],
                                      x8_3[:, ds(ko_lo, kos), :P])
                    nc.sync.dma_start(xt8[:, ds(ko_lo, kos), P:],
                                      x8_3[:, ds(ko_lo, kos), P:])
                else:
                    load_x8_krange(ko_lo, kos)
            for ko_lo, kos in chunks16:
                load_w16_krange(wq16_tiles[0], 0, ko_lo, kos)
                load_x16_krange(ko_lo, kos)
            bias_sb = bias_pool.tile([P, n_loc], mybir.dt.float32)
            nc.sync.dma_start(bias_sb[:], bb.ap())

            for nt in range(n_tiles):
                wq8t = wq8_tiles.pop(nt)
                wq16t = wq16_tiles.pop(nt)
                ps_tiles = [
                    psum_pool.tile([P, n_tile], mybir.dt.float32, tag="ps",
                                   name=f"ps{nt}_{mt}")
                    for mt in range(m_tiles)
                ]
                # ---- fp8 DoubleRow phase: 2 k-blocks per matmul
                for ci, (lo, sz) in enumerate(chunks8):
                    if nt + 1 < n_tiles:
                        if ci == 0:
                            wq8_tiles[nt + 1] = wq8_pool.tile(
                                [P, ko8_n, n_tile], mybir.dt.float8e4,
                                tag="wq8", name=f"wq8_{nt + 1}")
                        load_w8_krange(wq8_tiles[nt + 1], nt + 1, lo, sz)
                    for mt in range(m_tiles):
                        for ko in range(lo, lo + sz, 2):
                            nc.tensor.matmul(
                                ps_tiles[mt][:],
                                xt8[:, ds(ko, 2), ds(mt * P, P)],
                                wq8t[:, ds(ko, 2), :],
                                start=(ko == 0),
                                stop=False,
                                perf_mode=DR,
                            )
                # ---- fp16 phase
                for ci, (lo, sz) in enumerate(chunks16):
                    if nt + 1 < n_tiles:
                        if ci == 0:
                            wq16_tiles[nt + 1] = wq16_pool.tile(
                                [P, ko16_n, n_tile], mybir.dt.float16,
                                tag="wq16", name=f"wq16_{nt + 1}")
                        load_w16_krange(wq16_tiles[nt + 1], nt + 1, lo, sz)
                    last_chunk = ci == len(chunks16) - 1
                    for mt in range(m_tiles):
                        for ko in range(lo, lo + sz):
                            nc.tensor.matmul(
                                ps_tiles[mt][:],
                                xt16[:, ko, ds(mt * P, P)],
                                wq16t[:, ko, :],
                                start=False,
                                stop=(ko == ko16_n - 1),
                            )
                        if last_chunk:
                            yt = out_pool.tile([P, n_tile], mybir.dt.float32,
                                               tag="yt")
                            nc.vector.tensor_add(
                                yt[:], ps_tiles[mt][:],
                                bias_sb[:, ds(nt * n_tile, n_tile)])
                            nc.sync.dma_start(
                                y3[:, mt, ds(nt * n_tile, n_tile)], yt[:])

    nc.compile()
    return nc


_NC_CACHE = {}


def _get_nc():
    if "nc" not in _NC_CACHE:
        _NC_CACHE["nc"] = build_nc()
    return _NC_CACHE["nc"]


M_LOC = B_FULL // R
N_LOC = N_FULL // C
N_TILE = 512


def wire_x8(x_cols):
    """[m, K8] f32 -> partition-major [P, ko8, m] fp8 E4M3."""
    m, k = x_cols.shape
    return np.ascontiguousarray(
        x_cols.reshape(m, k // P, P).transpose(2, 1, 0)).astype(
            ml_dtypes.float8_e4m3)


def wire_x16(x_cols):
    """[m, K16] f32 -> partition-major [P, ko16, m] fp16."""
    m, k = x_cols.shape
    return np.ascontiguousarray(
        x_cols.reshape(m, k // P, P).transpose(2, 1, 0)).astype(np.float16)


def wire_w8(w_rows, n_tile=N_TILE):
    """[K8, n] f32 -> partition-major [P, nt, ko8, n_tile] u8 (f32 top byte)."""
    k, n = w_rows.shape
    hi = (np.ascontiguousarray(w_rows).view(np.uint32) >> 24).astype(np.uint8)
    return np.ascontiguousarray(
        hi.reshape(k // P, P, n // n_tile, n_tile).transpose(1, 2, 0, 3))


def wire_w16(w_rows, n_tile=N_TILE):
    """[K16, n] f32 -> partition-major [P, nt, ko16, n_tile] bf16."""
    k, n = w_rows.shape
    return np.ascontiguousarray(
        w_rows.reshape(k // P, P, n // n_tile, n_tile).transpose(1, 2, 0, 3)
    ).astype(ml_dtypes.bfloat16)


def wire_b(b_shard):
    """[n] f32 -> broadcast [P, n] f32."""
    return np.ascontiguousarray(
        np.broadcast_to(b_shard, (P, b_shard.shape[0])).astype(np.float32))


def make_in_maps(x, W, b):
    """Host-side shard + layout prep: per-core input dicts."""
    x = np.ascontiguousarray(np.asarray(x, dtype=np.float32))
    W = np.ascontiguousarray(np.asarray(W, dtype=np.float32))
    b = np.ascontiguousarray(np.asarray(b, dtype=np.float32))
    in_maps = []
    for core in range(N_CORES):
        i, j = divmod(core, C)
        xs = x[i * M_LOC:(i + 1) * M_LOC, :]
        ws = W[:, j * N_LOC:(j + 1) * N_LOC]
        in_maps.append({
            "xT8": wire_x8(xs[:, :K8]),
            "xT16": wire_x16(xs[:, K8:]),
            "w8": wire_w8(ws[:K8, :]),
            "w16": wire_w16(ws[K8:, :]),
            "bias": wire_b(b[j * N_LOC:(j + 1) * N_LOC]),
        })
    return in_maps


def gather_out(results):
    """Assemble per-core y shards into the full [4096, 4096] output."""
    y = np.empty((B_FULL, N_FULL), np.float32)
    for core in range(N_CORES):
        i, j = divmod(core, C)
        y[i * M_LOC:(i + 1) * M_LOC, j * N_LOC:(j + 1) * N_LOC] = (
            results[core]["y"])
    return y


def kernel(x, W, b):
    nc = _get_nc()
    in_maps = make_in_maps(x, W, b)
    res = bass_utils.run_bass_kernel_spmd(nc, in_maps,
                                          core_ids=list(range(N_CORES)))
    return gather_out(res.results)


# revision 23
# speedup vs baseline: 1.0067x; 1.0067x over previous
"""Binary dense layer  y = x @ sign(W) + b  on 8 Trainium2 NeuronCores.

Problem (hardcoded): x [4096, 4096] f32, W [4096, 4096] f32, b [4096] f32.

Sharding: 2D grid, 4 batch shards x 2 column shards (one core each).
Per core:  y[1024, 2048] = x[1024, 4096] @ sign(W[4096, 2048]) + b[2048].

Mixed-precision split-K: the 4096 contraction is split K8=2304 (x cast to
fp8 E4M3, matmuls in fp8 DoubleRow mode -- 2 fp8 weights per PE cell,
256-deep contraction per instruction) + K16=1792 (x cast to fp16, normal
matmuls). sign(W) = +-1 is exact in both dtypes and products accumulate
exactly in f32 PSUM, so the only quantization error is the fp8 rounding
of x over K8 of the contraction: measured rel err 1.991e-2 (< 2e-2 gate;
fp8-only would be 2.65e-2, fp16-only 2.1e-4 but ~1.3x slower). The fp8
rounding happens on the host, so the device result matches the numpy
prediction of the error to ~6 digits.

Wire formats (host-side layout prep): x is pre-transposed to K-major and
pre-cast (fp8/fp16 -- the same casts the device would do); W ships as its
sign-carrying high bytes (f32 top byte, u8) for the fp8 part and bf16 for
the fp16 part; both are binarized to +-1 on device with fused DVE bitwise
ops. Bias is pre-broadcast to [128, n]. Host gathers the 8 output shards.
"""

import ml_dtypes
import numpy as np

import concourse.bass as bass
import concourse.mybir as mybir
import concourse.tile as tile
from concourse import bacc, bass_utils
from concourse.bass import ds

# ---- problem constants (fixed by the task; kernel.py must be self-contained)
B_FULL = 4096  # batch rows of x
K_FULL = 4096  # contraction dim (n_in)
N_FULL = 4096  # output cols (n_units)
R, C = 4, 2  # batch shards x column shards -> R*C = 8 cores
N_CORES = 8
P = 128

K8 = 2304  # contraction columns computed in fp8 E4M3 (DoubleRow)
K16 = K_FULL - K8  # contraction columns computed in fp16


def _chunks(n_blocks, target):
    out, lo = [], 0
    while lo < n_blocks:
        sz = min(target, n_blocks - lo)
        out.append((lo, sz))
        lo += sz
    return out


def build_nc(m_loc=B_FULL // R, k8=K8, k16=K16, n_loc=N_FULL // C,
             n_tile=512):
    """Build + compile the per-core Bass kernel (SPMD: same NEFF on all cores).

    y[m_loc, n_loc] = x[m_loc, :] @ sign(W[:, n_loc]) + b[n_loc]

    Loop order is k-outer within each n-tile: all m_tile psum groups
    accumulate in lockstep over k-chunks (fp8 DoubleRow chunks first, then
    fp16 chunks), so during the prologue the PE computes on each arriving
    x k-chunk + W k-slice immediately instead of waiting for whole shards.
    """
    ko8_n = k8 // P
    ko16_n = k16 // P
    m_tiles = m_loc // P
    n_tiles = n_loc // n_tile
    # fp8 chunk sizes must be even (DoubleRow pairs). Fine-grained chunks at
    # the START: each chunk's binarize only fires when the WHOLE chunk's DMA
    # lands, so small early chunks let the first matmul pairs start while
    # later data is still in flight (the k-axis analog of the first-x m-split)
    chunks8 = [(0, 2), (2, 2), (4, 2)] + [
        (lo + 6, sz) for lo, sz in _chunks(ko8_n - 6, 4)]
    # any short remainder chunk goes FIRST: the LAST fp16 chunk runs the
    # per-mt epilogue (bias-add + store), which needs a full 4 matmuls per
    # m-tile to hide the DVE adds behind
    sizes16 = sorted((sz for _, sz in _chunks(ko16_n, 4)))
    chunks16, lo = [], 0
    for sz in sizes16:
        chunks16.append((lo, sz))
        lo += sz
    c8max = max(sz for _, sz in chunks8)
    c16max = max(sz for _, sz in chunks16)

    nc = bacc.Bacc("TRN2", target_bir_lowering=False, debug=False)

    # wire formats are partition-major (host pre-swizzled) so each DMA row
    # is a long contiguous run -> few, large DMA descriptors
    xT8 = nc.dram_tensor("xT8", [P, ko8_n, m_loc], mybir.dt.float8e4,
                         kind="ExternalInput")
    xT16 = nc.dram_tensor("xT16", [P, ko16_n, m_loc], mybir.dt.float16,
                          kind="ExternalInput")
    # W's fp8 half arrives as the top byte of each f32 (sign + 7 exponent
    # bits): sign-preserving and 1/4 the DMA bytes. The fp16 half arrives
    # as bf16 (sign-preserving cast).
    w8 = nc.dram_tensor("w8", [P, n_tiles, ko8_n, n_tile], mybir.dt.uint8,
                        kind="ExternalInput")
    w16 = nc.dram_tensor("w16", [P, n_tiles, ko16_n, n_tile],
                         mybir.dt.bfloat16, kind="ExternalInput")
    bb = nc.dram_tensor("bias", [P, n_loc], mybir.dt.float32,
                        kind="ExternalInput")
    y = nc.dram_tensor("y", [m_loc, n_loc], mybir.dt.float32,
                       kind="ExternalOutput")

    x8_3 = xT8.ap()
    x16_3 = xT16.ap()
    w8_4 = w8.ap()
    w16_4 = w16.ap()
    # output view: row index (mo*P + p) -> [p, mo, n]
    y3 = y.ap().rearrange("(mo p) n -> p mo n", p=P)

    DR = mybir.MatmulPerfMode.DoubleRow

    with tile.TileContext(nc) as tc:
        with (
            tc.tile_pool(name="x_res", bufs=1) as x_res_pool,
            tc.tile_pool(name="stage8", bufs=4) as stage8_pool,
            tc.tile_pool(name="stage16", bufs=4) as stage16_pool,
            tc.tile_pool(name="wq8", bufs=2) as wq8_pool,
            tc.tile_pool(name="wq16", bufs=2) as wq16_pool,
            tc.tile_pool(name="bias_sb", bufs=1) as bias_pool,
            tc.tile_pool(name="yout", bufs=4) as out_pool,
            tc.tile_pool(name="psum", bufs=8, space="PSUM") as psum_pool,
        ):
            # resident x shards, K on partitions
            xt8 = x_res_pool.tile([P, ko8_n, m_loc], mybir.dt.float8e4,
                                  name="xt8")
            xt16 = x_res_pool.tile([P, ko16_n, m_loc], mybir.dt.float16,
                                   name="xt16")

            def load_x8_krange(ko_lo, kos):
                nc.sync.dma_start(
                    xt8[:, ds(ko_lo, kos), :], x8_3[:, ds(ko_lo, kos), :])

            def load_x16_krange(ko_lo, kos):
                nc.sync.dma_start(
                    xt16[:, ds(ko_lo, kos), :], x16_3[:, ds(ko_lo, kos), :])

            def load_w8_krange(wq, nt, ko_lo, kos):
                # ko-range [P, kos, n_tile] u8 of the nt-th W column tile
                wstage = stage8_pool.tile([P, c8max, n_tile],
                                          mybir.dt.uint8, tag="w8stage",
                                          name=f"w8s{nt}_{ko_lo}")
                wst = wstage[:, :kos, :]
                nc.sync.dma_start(wst, w8_4[:, nt, ds(ko_lo, kos), :])
                # DVE fused bitwise binarize on u16 views (bit-parallel over
                # byte pairs): (w & 0x8080) | 0x3838 == +-1.0 fp8e4 per byte.
                dst = wq[:, ds(ko_lo, kos), :]
                nc.vector.tensor_scalar(
                    dst.bitcast(mybir.dt.uint16),
                    wst.bitcast(mybir.dt.uint16),
                    0x8080, 0x3838,
                    mybir.AluOpType.bitwise_and,
                    mybir.AluOpType.bitwise_or)

            def load_w16_krange(wq, nt, ko_lo, kos):
                wstage = stage16_pool.tile([P, c16max, n_tile],
                                           mybir.dt.bfloat16, tag="w16stage",
                                           name=f"w16s{nt}_{ko_lo}")
                wst = wstage[:, :kos, :]
                nc.sync.dma_start(wst, w16_4[:, nt, ds(ko_lo, kos), :])
                # (w & 0x8000) | 0x3C00 on u16 views == +-1.0 fp16
                dst = wq[:, ds(ko_lo, kos), :]
                nc.vector.tensor_scalar(
                    dst.bitcast(mybir.dt.uint16),
                    wst.bitcast(mybir.dt.uint16),
                    0x8000, 0x3C00,
                    mybir.AluOpType.bitwise_and,
                    mybir.AluOpType.bitwise_or)

            # PE warmup: scratch matmuls keep the PE busy through the DMA
            # prologue so the HAM clock gate flips to 8/8 as early as
            # possible (a free-running ~3.4us activity window); just enough
            # of them to bridge until the first real operands land -- every
            # extra warmup matmul delays the PE queue by ~0.5us of cold rate
            n_warm = 7
            scratch = x_res_pool.tile([P, n_tile], mybir.dt.float16,
                                      name="warm_scratch")
            nc.vector.memset(scratch[:], 0.0)
            ps_warm = psum_pool.tile([P, n_tile], mybir.dt.float32, tag="ps",
                                     name="ps_warm")
            for i in range(n_warm):
                nc.tensor.matmul(ps_warm[:], scratch[:, :P], scratch[:],
                                 start=(i == 0), stop=(i == n_warm - 1))

            # prologue: interleave x k-ranges with W tile 0 k-slices in
            # exactly the order the k-outer loop consumes them; the first
            # slice is split so the first real matmul starts sooner.
            # W goes first in each pair: its consumer chain (DMA ->
            # binarize -> MM) is longer than x's (DMA -> MM), and HWDGE
            # DMAs drain in FIFO order.
            wq8_tiles = {0: wq8_pool.tile([P, ko8_n, n_tile],
                                          mybir.dt.float8e4, tag="wq8",
                                          name="wq8_0")}
            wq16_tiles = {0: wq16_pool.tile([P, ko16_n, n_tile],
                                            mybir.dt.float16, tag="wq16",
                                            name="wq16_0")}
            lo0, sz0 = chunks8[0]
            ranges8 = ([(lo0, 2), (lo0 + 2, sz0 - 2)] if sz0 > 2
                       else [(lo0, sz0)]) + chunks8[1:]
            for idx, (ko_lo, kos) in enumerate(ranges8):
                load_w8_krange(wq8_tiles[0], 0, ko_lo, kos)
                if idx == 0:
                    # split the first x range by m so the very first matmul
                    # (m-tile 0) only waits on a 32 KB transfer
                    nc.sync.dma_start(xt8[:, ds(ko_lo, kos), :P],
                                      x8_3[:, ds(ko_lo, kos), :P])
                    nc.sync.dma_start(xt8[:, ds(ko_lo, kos), P:],
                                      x8_3[:, ds(ko_lo, kos), P:])
                else:
                    load_x8_krange(ko_lo, kos)
            for ko_lo, kos in chunks16:
                load_w16_krange(wq16_tiles[0], 0, ko_lo, kos)
                load_x16_krange(ko_lo, kos)
            bias_sb = bias_pool.tile([P, n_loc], mybir.dt.float32)
            nc.sync.dma_start(bias_sb[:], bb.ap())

            for nt in range(n_tiles):
                wq8t = wq8_tiles.pop(nt)
                wq16t = wq16_tiles.pop(nt)
                ps_tiles = [
                    psum_pool.tile([P, n_tile], mybir.dt.float32, tag="ps",
                                   name=f"ps{nt}_{mt}")
                    for mt in range(m_tiles)
                ]
                # ---- fp8 DoubleRow phase: 2 k-blocks per matmul
                for ci, (lo, sz) in enumerate(chunks8):
                    if nt + 1 < n_tiles:
                        if ci == 0:
                            wq8_tiles[nt + 1] = wq8_pool.tile(
                                [P, ko8_n, n_tile], mybir.dt.float8e4,
                                tag="wq8", name=f"wq8_{nt + 1}")
                        load_w8_krange(wq8_tiles[nt + 1], nt + 1, lo, sz)
                    for mt in range(m_tiles):
                        for ko in range(lo, lo + sz, 2):
                            nc.tensor.matmul(
                                ps_tiles[mt][:],
                                xt8[:, ds(ko, 2), ds(mt * P, P)],
                                wq8t[:, ds(ko, 2), :],
                                start=(ko == 0),
                                stop=False,
                                perf_mode=DR,
                            )
                # ---- fp16 phase
                for ci, (lo, sz) in enumerate(chunks16):
                    if nt + 1 < n_tiles:
                        if ci == 0:
                            wq16_tiles[nt + 1] = wq16_pool.tile(
                                [P, ko16_n, n_tile], mybir.dt.float16,
                                tag="wq16", name=f"wq16_{nt + 1}")
                        load_w16_krange(wq16_tiles[nt + 1], nt + 1, lo, sz)
                    last_chunk = ci == len(chunks16) - 1
                    for mt in range(m_tiles):
                        for ko in range(lo, lo + sz):
                            nc.tensor.matmul(
                                ps_tiles[mt][:],
                                xt16[:, ko, ds(mt * P, P)],
                                wq16t[:, ko, :],
                                start=False,
                                stop=(ko == ko16_n - 1),
                            )
                        if last_chunk:
                            yt = out_pool.tile([P, n_tile], mybir.dt.float32,
                                               tag="yt")
                            nc.vector.tensor_add(
                                yt[:], ps_tiles[mt][:],
                                bias_sb[:, ds(nt * n_tile, n_tile)])
                            nc.sync.dma_start(
                                y3[:, mt, ds(nt * n_tile, n_tile)], yt[:])

    nc.compile()
    return nc


_NC_CACHE = {}


def _get_nc():
    if "nc" not in _NC_CACHE:
        _NC_CACHE["nc"] = build_nc()
    return _NC_CACHE["nc"]


M_LOC = B_FULL // R
N_LOC = N_FULL // C
N_TILE = 512


def wire_x8(x_cols):
    """[m, K8] f32 -> partition-major [P, ko8, m] fp8 E4M3."""
    m, k = x_cols.shape
    return np.ascontiguousarray(
        x_cols.reshape(m, k // P, P).transpose(2, 1, 0)).astype(
            ml_dtypes.float8_e4m3)


def wire_x16(x_cols):
    """[m, K16] f32 -> partition-major [P, ko16, m] fp16."""
    m, k = x_cols.shape
    return np.ascontiguousarray(
        x_cols.reshape(m, k // P, P).transpose(2, 1, 0)).astype(np.float16)


def wire_w8(w_rows, n_tile=N_TILE):
    """[K8, n] f32 -> partition-major [P, nt, ko8, n_tile] u8 (f32 top byte)."""
    k, n = w_rows.shape
    hi = (np.ascontiguousarray(w_rows).view(np.uint32) >> 24).astype(np.uint8)
    return np.ascontiguousarray(
        hi.reshape(k // P, P, n // n_tile, n_tile).transpose(1, 2, 0, 3))


def wire_w16(w_rows, n_tile=N_TILE):
    """[K16, n] f32 -> partition-major [P, nt, ko16, n_tile] bf16."""
    k, n = w_rows.shape
    return np.ascontiguousarray(
        w_rows.reshape(k // P, P, n // n_tile, n_tile).transpose(1, 2, 0, 3)
    ).astype(ml_dtypes.bfloat16)


def wire_b(b_shard):
    """[n] f32 -> broadcast [P, n] f32."""
    return np.ascontiguousarray(
        np.broadcast_to(b_shard, (P, b_shard.shape[0])).astype(np.float32))


def make_in_maps(x, W, b):
    """Host-side shard + layout prep: per-core input dicts."""
    x = np.ascontiguousarray(np.asarray(x, dtype=np.float32))
    W = np.ascontiguousarray(np.asarray(W, dtype=np.float32))
    b = np.ascontiguousarray(np.asarray(b, dtype=np.float32))
    in_maps = []
    for core in range(N_CORES):
        i, j = divmod(core, C)
        xs = x[i * M_LOC:(i + 1) * M_LOC, :]
        ws = W[:, j * N_LOC:(j + 1) * N_LOC]
        in_maps.append({
            "xT8": wire_x8(xs[:, :K8]),
            "xT16": wire_x16(xs[:, K8:]),
            "w8": wire_w8(ws[:K8, :]),
            "w16": wire_w16(ws[K8:, :]),
            "bias": wire_b(b[j * N_LOC:(j + 1) * N_LOC]),
        })
    return in_maps


def gather_out(results):
    """Assemble per-core y shards into the full [4096, 4096] output."""
    y = np.empty((B_FULL, N_FULL), np.float32)
    for core in range(N_CORES):
        i, j = divmod(core, C)
        y[i * M_LOC:(i + 1) * M_LOC, j * N_LOC:(j + 1) * N_LOC] = (
            results[core]["y"])
    return y


def kernel(x, W, b):
    nc = _get_nc()
    in_maps = make_in_maps(x, W, b)
    res = bass_utils.run_bass_kernel_spmd(nc, in_maps,
                                          core_ids=list(range(N_CORES)))
    return gather_out(res.results)
